# revision 39
# baseline (speedup 1.0000x reference)
"""
LutLinear (BCQ 3-bit, group=128) matvec kernel for 8 Trainium2 NeuronCores.

y = x @ W + bias,  W[k,n] = sum_b alpha[g(k),b,n]*B[k,b,n] + q_bias[g(k),n]
with B = 2*bit-1 from bit-packed binaryWeight [K//32, WBIT, N] (bit j of word
i <-> input index 32*i + j).

Strategy (tensor-parallel over N, 512 columns per core):
  y[n] = sum_{g,b} 2*alpha[g,b,n] * tbit[g,b,n] + bias_eff[n]
  tbit[g,b,n] = sum_{k in g} x_k * bit[k,b,n]
  bias_eff = bias + s @ (q_bias - sum_b alpha) - alpha-weighted OR-correction
  (all the constant terms are host-folded; s_g = per-group sums of x)

Pipeline (~24us median, vs 25.7us for the unchunked predecessor):
  - The words DMA is chunked by output bit-plane b (b0 split once more in
    half), so the DVE deposit stream starts as soon as the first ~131KB
    lands (~9.2-9.5us) instead of waiting for the full words tensor; each
    chunk's deposits+matmuls gate only on that chunk's semaphore.  All
    transfers ride one sync-HWDGE queue in consumption order.
  - Deposit pass (r, chunk) on DVE (the only engine with 32-bit bitwise
    ops, ~0.33us per 512-col chunk at 2x_2P): dep = (words & mask_r)
    [| 0x38383838]; byte lane L is an fp8e4m3 value a_r + d_r*bit.
    Plane r=7 for b2 only is host-precomputed and DMA'd (HOST_RS): the
    slowest SDMA engine (often idx 15, up to ~3x slow, start lag 0-2us)
    trails the aggregate stream by ~bytes/16/10GB/s, so total HBM traffic
    is kept to 1.41MB and nothing later than h7b2 gates the tail.  alpha
    (identical for the four L-quadrants) ships once as [32, F] and is
    replicated to partitions 32-127 by three SBUF->SBUF DMAs off-path;
    b2's final plane is deposited in halves so tb2's stop -- the tail's
    gate -- fires off a half-width block.
  - PE contracts words with accumulating fp8 DoublePixel matmuls (4
    L-quadrant streams per plane) into one PSUM accumulator per b.
    Engines execute their queues IN EMISSION ORDER, so every matmul block
    is emitted at the slot where its gating semaphore is expected to fire
    (a late block parked early head-of-line-blocks everything: -3.3us).
  - tb0/tb1 stop early (all-DVE groups) and exit PSUM via idle-ACT fp16
    copies, making their alpha multiplies cheap SBUF fp16 TTs; only b2's
    multiply reads PSUM on the critical tail.
  - The y reduction uses sliding-window one-hot stationaries (ew) to land
    the four 128-col chunk sums on PSUM partitions 0/4/8/12: the ACT
    copy-out runs partition-parallel (0.36us vs 0.68) and the out-DMA
    spreads over 4 SDMA engines.  Bias is folded in via K=1 matmuls that
    open the accumulation group; the group stop rides the last real
    matmul (the ACT copy's ~0.3us pipe-fill clears the PSUM drain), and
    the out-DMA is issued from the ACT engine itself (no cross-engine
    hop, separate queue from any input straggler backlog).
"""

import sys

import numpy as np

sys.path.insert(0, "/opt/trn_rl_repo")

import concourse.bacc as bacc
import concourse.bass as bass
import concourse.mybir as mybir
import concourse.tile as tile
from concourse.bass_utils import run_bass_kernel_spmd

K, N, WBIT, GROUP = 4096, 4096, 3, 128
NCORES = 8
NS = N // NCORES          # 512 output columns per core
NG = K // GROUP           # 32 groups
W = K // 32               # 128 packed words along K
F = WBIT * NS             # 1536 free elements (b, n) per partition
FP8_OR = 0x38383838       # 1.0 in every fp8e4m3 byte lane
OR_RS = (0, 1, 2, 7)
# host-DMA'd deposit planes per b-chunk (the rest are DVE tensor_scalar):
# b0 is fully on the DVE (its deposits run while the DMA queue is still
# words-heavy, and an all-DVE b0 stops tb0 early for the ACT-copy tail);
# the host planes serve the late b's, sized so the DMA stream drains by
# the time the DVE finishes.  Total DMA is kept under ~2MB: the slowest
# SDMA engine (often E79, up to ~3x slow) trails the aggregate stream by
# ~bytes/16/10GB/s, delaying every later semaphore and the exec-end drain.
HOST_RS = {0: (), 1: (), 2: (7,)}
D_R = {0: 0.125, 1: 0.25, 2: 0.5, 3: 2.0**-6, 4: 2.0**-5, 5: 2.0**-3,
       6: 2.0, 7: -2.0}

_CACHE = {}


def _declare_io(nc):
    f32 = mybir.dt.float32
    i32 = mybir.dt.int32
    fp16 = mybir.dt.float16
    d = {}
    # words split by b-plane; b0 further split in half so the DVE starts early
    d["w0a"] = nc.declare_dram_parameter("w0a", [W, NS // 2], i32, isOutput=False)
    d["w0b"] = nc.declare_dram_parameter("w0b", [W, NS // 2], i32, isOutput=False)
    d["w1"] = nc.declare_dram_parameter("w1", [W, NS], i32, isOutput=False)
    d["w2"] = nc.declare_dram_parameter("w2", [W, NS], i32, isOutput=False)
    d["xb"] = nc.declare_dram_parameter("xb", [W, 32 * 32], fp16, isOutput=False)
    d["alpha16"] = nc.declare_dram_parameter(
        "alpha16", [32, F], fp16, isOutput=False)
    d["bias16"] = nc.declare_dram_parameter("bias16", [1, NS], fp16, isOutput=False)
    for b, rs in HOST_RS.items():
        for r in rs:
            d[f"h{r}b{b}"] = nc.declare_dram_parameter(
                f"h{r}b{b}", [W, NS], i32, isOutput=False)
    d["y"] = nc.declare_dram_parameter("y", [16, NS // 4], f32, isOutput=True)
    return d


def _emit_body(nc, tiles, dram):
    f8 = mybir.dt.float8e4
    (words, xb, alpha, bias_t, dep, prod, tbs, ones, ew, junk_w, junk_m,
     y_sb) = tiles
    tb0, tb1, tb2, yp, junk_p = _CACHE["psum_tiles"]
    tbs_psum = (tb0, tb1, tb2)

    # all input DMAs on one (fanning-out) sync queue, ordered so the words
    # chunk gating each DVE deposit group arrives just before the DVE needs
    # it, with host deposit planes and alpha slotted into the spare slots
    # DMA queue order = consumption order: words chunks pace the DVE; the
    # host planes and alpha ride behind, each landing just before its
    # consumer block comes up in the (in-order) PE/DVE streams
    nc.sync.dma_start(words[:, 0:NS // 2], dram["w0a"][:])
    nc.sync.dma_start(words[:, NS // 2:NS], dram["w0b"][:])
    nc.sync.dma_start(xb[:], dram["xb"][:])
    nc.sync.dma_start(words[:, NS:2 * NS], dram["w1"][:])
    nc.sync.dma_start(words[:, 2 * NS:3 * NS], dram["w2"][:])
    nc.sync.dma_start(alpha[0:32, :], dram["alpha16"][:])
    nc.sync.dma_start(dep[:, 7 * F + 2 * NS:7 * F + 3 * NS], dram["h7b2"][:])
    nc.sync.dma_start(bias_t[:], dram["bias16"][:])
    # alpha rows repeat every 32 partitions (same alpha for each L
    # quadrant): ship it once and replicate via SBUF->SBUF DMAs -- 295KB
    # less HBM traffic through the straggler engine.  Issued from Sync
    # (idle after the input issues); on ACT they would park ahead of the
    # table-load prewarm and the tb copies.
    for q in range(1, 4):
        nc.sync.dma_start(alpha[32 * q:32 * (q + 1), :], alpha[0:32, :])
    nc.gpsimd.memset(junk_w[:], 1.0)
    nc.gpsimd.memset(junk_m[:], 0.0)
    nc.gpsimd.memset(ones[:], 1.0)
    # sliding-window one-hot stationary: ew[:, 12] = 1, rest 0; the slice
    # ew[:, 12-4c : 28-4c] is a [128, 16] matrix whose only ones-column is
    # 4c, so the yp16 reduction lands chunk c's sums on PSUM partition 4c
    # (partitions 0/4/8/12 -> four different SDMA engines on the way out)
    nc.gpsimd.memset(ew[:], 0.0)
    nc.gpsimd.memset(ew[:, 12:13], 1.0)
    # prewarm the ACT table set (Copy) so the first real ACT copy doesn't
    # pay the ~2.7us PSEUDO_LOAD_ACT_FUNC_SET on the critical path
    nc.scalar.copy(junk_w[0:1, 0:1], junk_w[0:1, 1:2])

    # warm-up matmuls fill the otherwise-idle PE window during the first
    # words-chunk DMA so the p-state ramp happens before the real blocks
    for _ in range(7):
        nc.tensor.matmul(
            junk_p[0:2, :], junk_w[:], junk_m[:],
            start=True, stop=True, skip_group_check=True,
        )

    dep8 = dep[:].bitcast(f8).rearrange("w (r i l) -> w r i l", r=8, i=F, l=4)

    def deposit(r, c0, c1):
        blk = dep[:, r * F + c0:r * F + c1]
        mask = (0x01010101 << r) & 0xFFFFFFFF
        if mask >= 1 << 31:
            mask -= 1 << 32
        if r in OR_RS:
            nc.vector.tensor_scalar(
                blk, words[:, c0:c1], mask, FP8_OR,
                op0=mybir.AluOpType.bitwise_and,
                op1=mybir.AluOpType.bitwise_or,
            )
        else:
            nc.vector.tensor_scalar(
                blk, words[:, c0:c1], mask, None,
                op0=mybir.AluOpType.bitwise_and,
            )

    def mm_block(r, b, start, stop, c0=0, c1=NS):
        for L in range(4):
            j = r + 8 * L
            nc.tensor.matmul(
                tbs_psum[b][32 * L:32 * (L + 1), c0:c1],
                xb[:, j * 32:(j + 1) * 32],
                dep8[:, r, b * NS + c0:b * NS + c1, L],
                start=start,
                stop=stop,
                tile_position=(0, 32 * L),
                perf_mode=mybir.MatmulPerfMode.DoublePixel,
                skip_group_check=True,
            )

    # Engines execute their streams IN EMISSION ORDER, so every block is
    # emitted at the slot where its gating data (deposit chunk or host-DMA
    # semaphore) is expected to arrive -- a late block parked early in the
    # stream head-of-line-blocks everything behind it (measured 3.3us).
    b0_rs = [r for r in range(8) if r not in HOST_RS[0]]
    b1_rs = [r for r in range(8) if r not in HOST_RS[1]]
    b2_rs = [r for r in range(8) if r not in HOST_RS[2]]
    # b0: the first two planes ride the half-size w0a transfer (earliest
    # possible DVE start); by plane r2 the w0b half has landed, so the
    # rest run full-width (a half-chunk pays +58 cyc instruction overhead)
    for i, r in enumerate(b0_rs):
        if i < 2:
            deposit(r, 0, NS // 2)
            deposit(r, NS // 2, NS)
        else:
            deposit(r, 0, NS)
        mm_block(r, 0, start=(i == 0), stop=(i == len(b0_rs) - 1))
    # b1 DVE planes (gated only on w1 -- merging with b2 into wider
    # deposits was measured 1.8us slower: it parks the whole stream on
    # w2's straggler-laden semaphore)
    for i, r in enumerate(b1_rs):
        deposit(r, NS, 2 * NS)
        mm_block(r, 1, start=(i == 0), stop=(i == len(b1_rs) - 1))
    # b2: DVE planes chase the deposit stream; the h7b2 host block slots
    # second-to-last (its DMA semaphore fires ~12-13us on a healthy run,
    # and a late slot avoids head-of-line blocking when the DMA runs slow)
    for i, r in enumerate(b2_rs):
        if r == b2_rs[-1]:
            # final plane in halves: tb2's stop (the tail's gate) rides a
            # half-width block and fires ~0.3us sooner
            deposit(r, 2 * NS, 2 * NS + NS // 2)
            mm_block(7, 2, start=False, stop=False)  # h7b2-gated
            mm_block(r, 2, start=False, stop=False, c0=0, c1=NS // 2)
            deposit(r, 2 * NS + NS // 2, 3 * NS)
            mm_block(r, 2, start=False, stop=True, c0=NS // 2, c1=NS)
        else:
            deposit(r, 2 * NS, 3 * NS)
            mm_block(r, 2, start=(i == 0), stop=False)
    # fold the bias into yp16 via K=1 matmuls opening the accumulation
    # group: row 4c of yp16 gets bias chunk c
    for c in range(4):
        nc.tensor.matmul(
            yp[:], ew[0:1, 12 - 4 * c:28 - 4 * c],
            bias_t[0:1, 128 * c:128 * (c + 1)],
            start=(c == 0), stop=False, skip_group_check=True,
        )

    # tails: b0/b1 exit PSUM via the idle ACT engine (fp16 cast) as soon
    # as their (early, all-DVE) stops fire, so their alpha multiplies are
    # cheap SBUF fp16 TTs; b2 multiplies straight out of PSUM last, right
    # as its stop fires -- TT order (0, 1, 2) matches gate arrival.
    for b in (0, 1):
        nc.scalar.copy(tbs[b][:], tbs_psum[b][:])
    for b in (0, 1, 2):
        sl = slice(b * NS, (b + 1) * NS)
        src = tbs[b][:] if b < 2 else tbs_psum[b][:]
        nc.vector.tensor_tensor(
            prod[:, sl], src, alpha[:, sl],
            op=mybir.AluOpType.mult)
    # stop rides the last yp matmul: the ACT copy's ~0.3us table-lookup
    # pipe-fill before it reads column 0 clears the ~100ns PSUM drain
    for b in (0, 1, 2):
        for c in range(4):
            nc.tensor.matmul(
                yp[:], ew[:, 12 - 4 * c:28 - 4 * c],
                prod[:, b * NS + 128 * c:b * NS + 128 * (c + 1)],
                start=False, stop=(b == 2 and c == 3),
                skip_group_check=True,
            )
    # copy-out on ACT (bias already accumulated in yp16); the out-DMA is
    # issued from the same engine so no cross-engine semaphore hop, and it
    # rides the Q10 ring, clear of any input-queue straggler backlog
    nc.scalar.copy(y_sb[:], yp[:])
    nc.scalar.dma_start(dram["y"][:], y_sb[:])


def _build_program():
    nc = bacc.Bacc(None, target_bir_lowering=False, debug=False)
    f32 = mybir.dt.float32
    i32 = mybir.dt.int32
    fp16 = mybir.dt.float16
    dram = _declare_io(nc)

    with tile.TileContext(nc) as tc:
        with (
            tc.tile_pool(name="pool", bufs=1) as pool,
            tc.tile_pool(name="psum", bufs=1, space="PSUM") as psum,
        ):
            words = pool.tile([W, F], i32, name="words_sb")
            xb = pool.tile([W, 32 * 32], fp16, name="xb_sb")
            alpha = pool.tile([128, F], fp16, name="alpha_sb")
            bias_t = pool.tile([1, NS], fp16, name="bias_sb")
            dep = pool.tile([W, 8 * F], i32, name="dep_sb")
            prod = pool.tile([128, F], fp16, name="prod_sb")
            tbs = tuple(
                pool.tile([128, NS], fp16, name=f"tbs{b}_sb") for b in range(2))
            ones = pool.tile([128, 1], fp16, name="ones_sb")
            ew = pool.tile([128, 28], fp16, name="ew_sb")
            junk_w = pool.tile([128, 2], fp16, name="junkw_sb")
            junk_m = pool.tile([128, NS], fp16, name="junkm_sb")
            y_sb = pool.tile([16, NS // 4], f32, name="y_out_sb")
            _CACHE["psum_tiles"] = (
                psum.tile([128, NS], f32, name="tb0"),
                psum.tile([128, NS], f32, name="tb1"),
                psum.tile([128, NS], f32, name="tb2"),
                psum.tile([16, NS // 4], f32, name="yp"),
                psum.tile([128, NS], f32, name="junk_p"),
            )
            tiles = (words, xb, alpha, bias_t, dep, prod, tbs, ones, ew,
                     junk_w, junk_m, y_sb)
            _emit_body(nc, tiles, dram)

    nc.compile()
    return nc


def _host_prep(x, binaryWeight, alpha, q_bias, bias):
    """Build the 8 per-core input maps (pure layout work + tiny matvecs)."""
    x = np.asarray(x, np.float32).reshape(K)
    bw = np.asarray(binaryWeight)            # [W, WBIT, N] int32
    al = np.asarray(alpha, np.float32)       # [NG, WBIT, N]
    qb = np.asarray(q_bias, np.float32)      # [NG, N]
    bs = np.asarray(bias, np.float32)        # [N]

    xv = x.reshape(W, 32)                                # [w, j]
    dscale = np.array([D_R[j % 8] for j in range(32)], np.float32)
    blk = np.zeros((W, 32, 32), np.float16)              # [w, j, g]
    for g in range(32):
        blk[4 * g:4 * g + 4, :, g] = (
            xv[4 * g:4 * g + 4, :] / dscale[None, :]).astype(np.float16)
    # per-(32L+g) sum of OR'd-pass stationary values: the a_r=1.0 byte offset
    # contributes corr[p] * alpha16[p, f] to tb, folded into bias_eff below
    corr = np.zeros(128, np.float32)
    for L in range(4):
        for g in range(32):
            corr[32 * L + g] = sum(
                blk[4 * g:4 * g + 4, r + 8 * L, g].astype(np.float32).sum()
                for r in OR_RS)
    s = x.reshape(NG, GROUP).sum(axis=1).astype(np.float32)   # [NG]
    asum = al.sum(axis=1)                                # [NG, N]
    bias_eff = bs + s @ (qb - asum)                      # [N]
    xb_full = np.ascontiguousarray(blk.reshape(W, 32 * 32))

    in_maps = []
    for c in range(NCORES):
        n0 = c * NS
        words = np.ascontiguousarray(
            bw[:, :, n0:n0 + NS].reshape(W, F)).astype(np.int32)
        alpha_s = (2.0 * al[:, :, n0:n0 + NS].reshape(NG, F)).astype(np.float16)
        alpha16 = np.ascontiguousarray(alpha_s)
        # alpha-weighted OR-correction, exact in f32, folded into the bias
        a32 = alpha16.astype(np.float32)
        ycorr = sum((corr[32 * L:32 * (L + 1), None] * a32).sum(axis=0)
                    for L in range(4))
        ycorr = ycorr.reshape(WBIT, NS).sum(axis=0)      # [NS]
        m = {
            "w0a": np.ascontiguousarray(words[:, 0:NS // 2]),
            "w0b": np.ascontiguousarray(words[:, NS // 2:NS]),
            "w1": np.ascontiguousarray(words[:, NS:2 * NS]),
            "w2": np.ascontiguousarray(words[:, 2 * NS:3 * NS]),
            "xb": xb_full,
            "alpha16": alpha16,
            "bias16": np.ascontiguousarray(
                (bias_eff[n0:n0 + NS] - ycorr).astype(np.float16).reshape(1, NS)),
        }
        wu = words.view(np.uint32)
        for b, rs in HOST_RS.items():
            for r in rs:
                v = wu[:, b * NS:(b + 1) * NS] & np.uint32(
                    (0x01010101 << r) & 0xFFFFFFFF)
                if r in OR_RS:
                    v = v | np.uint32(0x38383838)
                m[f"h{r}b{b}"] = np.ascontiguousarray(v.view(np.int32))
        in_maps.append(m)
    return in_maps


def kernel(x, binaryWeight, alpha, q_bias, bias, _trace=False):
    if "nc" not in _CACHE:
        _CACHE["nc"] = _build_program()
    nc = _CACHE["nc"]
    in_maps = _host_prep(x, binaryWeight, alpha, q_bias, bias)
    res = run_bass_kernel_spmd(nc, in_maps, list(range(NCORES)), trace=_trace)
    _CACHE["last_res"] = res
    _CACHE["last_exec_time_ns"] = res.exec_time_ns
    # y arrives as [16, 128] with chunk c of the 512 columns on row 4c
    y = np.concatenate(
        [res.results[c]["y"][[0, 4, 8, 12], :].reshape(1, NS)
         for c in range(NCORES)], axis=1)
    return y.astype(np.float32)
